# revision 7
# baseline (speedup 1.0000x reference)
"""Trainium2 Bass kernel for nn_Discriminator (W_down projection + time-embed
+ W_vt/W_ih projections + 16-step LSTM + linear head).

Strategy: pure data-parallel over batch B=128 across 8 NeuronCores (16
batches/core), all weights replicated. Heavy operands are pre-transposed,
padded, and cast to fp16 on host (layout/precision prep only — all contractions
run on-device with fp32 PSUM accumulation). Everything on-device is laid out
feature-on-partition ("T layout") so the LSTM elementwise ops run on full 128
partitions:

  vT[n, r]      : [20480, 256]  r = l*16 + b (l-major rows per core)
  vdT = WdT.T @ vT   (contraction n, accumulated over 160 K-tiles)
  teT = wt (x) t + wtb (rank-1 matmuls, K=1)
  inputsT = WvtT.T @ [vdT; teT]
  xgT = WihT.T @ inputsT + (b_ih + b_hh)      [4096, 256] fp16, SBUF resident
  LSTM (weight-stationary, gates transposed):
     gatesT_t = xgT[:, t] + sum_k WhhT[k].T @ hT[k]  in two [128,256] PSUM halves
     i,f,g,o on [128,128] tiles; cT/hT as [128, 8*16]
  pred = hT.T @ lin_wT + lin_b  -> [16, 1] per core
"""

import numpy as np

import concourse.bass as bass
import concourse.bacc as bacc
import concourse.tile as tile
from concourse import mybir
from concourse.bass_utils import run_bass_kernel_spmd

F32 = mybir.dt.float32
F16 = mybir.dt.float16

B, L, N = 128, 16, 20000
HIN, H, HT = 512, 1024, 128
G = 4 * H                     # 4096 gate rows
NCORES = 8
BLOC = B // NCORES            # 16 batches per core
R = BLOC * L                  # 256 rows per core (l-major)
P = 128
NCH = 8                       # contraction tiles per DMA chunk
NT = ((N + P - 1) // P + NCH - 1) // NCH * NCH   # 160 contraction tiles
NCHUNKS = NT // NCH           # 20 DMA chunks
NPAD = NT * P                 # 20480
DT = HIN // P                 # 4 vd tiles
KVT = (HIN + HT) // P         # 5 vt contraction tiles
MC = G // P                   # 32 gate row-tiles
KC = HIN // P                 # 4 xg contraction tiles
KH = H // P                   # 8 lstm contraction tiles

_CACHE = {}


def _build(phases=("A", "B", "C", "L"), null=False, reps=1, cache=True):
    """Build + compile the SPMD Bass module once.

    phases/null/reps are ablation & timing knobs for performance experiments;
    the graded path always builds the full kernel with reps=1.
    """
    key = (tuple(phases), null, reps)
    if cache and key in _CACHE:
        return _CACHE[key]

    nc = bacc.Bacc("TRN2", target_bir_lowering=False, debug=False,
                   num_devices=NCORES)

    d_vT = nc.dram_tensor("vT", [NPAD, R], F16, kind="ExternalInput")
    d_WdT = nc.dram_tensor("WdT", [NPAD, HIN], F16, kind="ExternalInput")
    d_WvtT = nc.dram_tensor("WvtT", [HIN + HT, HIN], F16, kind="ExternalInput")
    d_WihT = nc.dram_tensor("WihT", [HIN, G], F16, kind="ExternalInput")
    d_WhhT = nc.dram_tensor("WhhT", [H, G], F16, kind="ExternalInput")
    d_t = nc.dram_tensor("t_row", [1, R], F32, kind="ExternalInput")
    d_wt = nc.dram_tensor("wt_row", [1, HT], F32, kind="ExternalInput")
    d_wtb = nc.dram_tensor("wtb_row", [1, HT], F32, kind="ExternalInput")
    d_bias = nc.dram_tensor("bias_g", [P, MC], F32, kind="ExternalInput")
    d_linw = nc.dram_tensor("lin_wT", [P, KH], F16, kind="ExternalInput")
    d_linb = nc.dram_tensor("lin_b_col", [BLOC, 1], F32, kind="ExternalInput")
    d_pred = nc.dram_tensor("pred", [BLOC, 1], F32, kind="ExternalOutput")

    SIG = mybir.ActivationFunctionType.Sigmoid
    TANH = mybir.ActivationFunctionType.Tanh
    IDENT = mybir.ActivationFunctionType.Identity

    with tile.TileContext(nc) as tc:
        with (
            tc.tile_pool(name="const", bufs=1) as const,
            tc.tile_pool(name="vstream", bufs=3) as vpool,
            tc.tile_pool(name="wdstream", bufs=3) as wdpool,
            tc.tile_pool(name="ws", bufs=2) as ws,
            tc.tile_pool(name="h16", bufs=2) as h16pool,
            tc.tile_pool(name="psmm", bufs=1, space="PSUM") as psmm,
        ):
            if null:
                linb_sb = const.tile([BLOC, 1], F32)
                nc.sync.dma_start(out=linb_sb, in_=d_linb[:])
                pred_sb = const.tile([BLOC, 1], F32)
                nc.scalar.copy(out=pred_sb, in_=linb_sb)
                nc.sync.dma_start(out=d_pred[:], in_=pred_sb)
            else:
                _emit(nc, tc, const, vpool, wdpool, ws, h16pool, psmm,
                      phases, reps,
                      d_vT, d_WdT, d_WvtT, d_WihT, d_WhhT, d_t, d_wt, d_wtb,
                      d_bias, d_linw, d_linb, d_pred, SIG, TANH, IDENT)

    nc.compile()
    if cache:
        _CACHE[key] = nc
    return nc


def _emit(nc, tc, const, vpool, wdpool, ws, h16pool, psmm, phases, reps,
          d_vT, d_WdT, d_WvtT, d_WihT, d_WhhT, d_t, d_wt, d_wtb,
          d_bias, d_linw, d_linb, d_pred, SIG, TANH, IDENT):
    # ---- early constants (small) ----
    wvt_sb = const.tile([P, KVT * HIN], F16)   # 5KB/part
    for k in range(KVT):
        nc.sync.dma_start(out=wvt_sb[:, k * HIN:(k + 1) * HIN],
                          in_=d_WvtT[k * P:(k + 1) * P, :])
    bias_sb = const.tile([P, MC], F32)
    nc.sync.dma_start(out=bias_sb, in_=d_bias[:])
    linw_sb = const.tile([P, KH], F16)
    nc.sync.dma_start(out=linw_sb, in_=d_linw[:])
    linb_sb = const.tile([BLOC, 1], F32)
    nc.sync.dma_start(out=linb_sb, in_=d_linb[:])
    t_sb = const.tile([1, R], F32)
    nc.sync.dma_start(out=t_sb, in_=d_t[:])
    wt_sb = const.tile([1, HT], F32)
    nc.sync.dma_start(out=wt_sb, in_=d_wt[:])
    wtb_sb = const.tile([1, HT], F32)
    nc.sync.dma_start(out=wtb_sb, in_=d_wtb[:])
    ones_sb = const.tile([1, R], F32)
    nc.vector.memset(ones_sb, 1.0)

    vt_sb = const.tile([P, KVT * R], F16)      # vdT + teT
    inpT_sb = const.tile([P, KC * R], F16)     # inputsT
    xgT_sb = const.tile([P, MC * R], F16)      # 16KB/part
    cT = const.tile([P, HT], F32)              # cell state [128, 8*16]

    whh_sb = const.tile([P, KH * G], F16)      # 64KB/part
    wih_sb = const.tile([P, KC * G], F16)      # 32KB/part

    wdT3 = d_WdT.rearrange("(a p) d -> p a d", p=P)   # [128,160,512]
    vT3 = d_vT.rearrange("(a p) r -> p a r", p=P)     # [128,160,256]
    xg3 = xgT_sb.rearrange("p (m r) -> p m r", m=MC)  # [128, 32, 256]

    for rep in range(reps):
        # ---- phase A: vdT[d, r] += WdT[n,d].T @ vT[n,r] ----
        # 8 K-tiles per DMA (HWDGE per-dma_start cost amortized)
        psA = [psmm.tile([P, R], F32, tag=f"psA{d}", name=f"psA{d}")
               for d in range(DT)]
        for c in range(NCHUNKS if "A" in phases else 0):
            wd_t = wdpool.tile([P, NCH, HIN], F16, tag="wd",
                               name="wd_t", bufs=3)
            nc.sync.dma_start(out=wd_t,
                              in_=wdT3[:, c * NCH:(c + 1) * NCH, :])
            v_t = vpool.tile([P, NCH, R], F16, tag="v", name="v_t",
                             bufs=3)
            nc.sync.dma_start(out=v_t,
                              in_=vT3[:, c * NCH:(c + 1) * NCH, :])
            for a in range(NCH):
                n = c * NCH + a
                for d in range(DT):
                    nc.tensor.matmul(
                        psA[d], lhsT=wd_t[:, a, d * P:(d + 1) * P],
                        rhs=v_t[:, a, :],
                        start=(n == 0), stop=(n == NT - 1))

        if rep == 0:
            # late preloads: needed from phase C onward; emitted after the
            # phase-A stream so they fill DMA gaps instead of delaying it
            for k in range(KH):
                nc.sync.dma_start(out=whh_sb[:, k * G:(k + 1) * G],
                                  in_=d_WhhT[k * P:(k + 1) * P, :])
            for k in range(KC):
                nc.sync.dma_start(out=wih_sb[:, k * G:(k + 1) * G],
                                  in_=d_WihT[k * P:(k + 1) * P, :])

        # te: wt[j] * t[r] + wtb[j]  (rank-1 updates, K=1)
        psTE = psmm.tile([P, R], F32, tag="psL0", name="psTE", bufs=2)
        nc.tensor.matmul(psTE, lhsT=wt_sb, rhs=t_sb, start=True, stop=False)
        nc.tensor.matmul(psTE, lhsT=wtb_sb, rhs=ones_sb, start=False,
                         stop=True)
        if "A" in phases:
            for d in range(DT):
                if d % 2 == 0:
                    nc.vector.tensor_copy(out=vt_sb[:, d * R:(d + 1) * R],
                                          in_=psA[d])
                else:
                    nc.scalar.copy(out=vt_sb[:, d * R:(d + 1) * R],
                                   in_=psA[d])
        nc.vector.tensor_copy(out=vt_sb[:, DT * R:(DT + 1) * R], in_=psTE)

        # ---- phase B: inputsT[i, r] ----
        for m in range(KC if "B" in phases else 0):
            psB = psmm.tile([P, R], F32, tag=f"psA{m}", name=f"psB{m}")
            for k in range(KVT):
                nc.tensor.matmul(
                    psB,
                    lhsT=wvt_sb[:, k * HIN + m * P:k * HIN + (m + 1) * P],
                    rhs=vt_sb[:, k * R:(k + 1) * R],
                    start=(k == 0), stop=(k == KVT - 1))
            nc.vector.tensor_copy(out=inpT_sb[:, m * R:(m + 1) * R],
                                  in_=psB)

        # ---- phase C: xgT[g, r] = WihT.T @ inputsT + bias ----
        for m in range(MC if "C" in phases else 0):
            psC = psmm.tile([P, R], F32, tag=f"psA{m % DT}", name=f"psC{m}")
            for k in range(KC):
                nc.tensor.matmul(
                    psC,
                    lhsT=wih_sb[:, k * G + m * P:k * G + (m + 1) * P],
                    rhs=inpT_sb[:, k * R:(k + 1) * R],
                    start=(k == 0), stop=(k == KC - 1))
            nc.scalar.activation(out=xgT_sb[:, m * R:(m + 1) * R], in_=psC,
                                 func=IDENT, bias=bias_sb[:, m:m + 1],
                                 scale=1.0)

        # ---- LSTM over L steps ----
        h_prev = None
        for t in range(L if "L" in phases else 0):
            acts = []
            if t == 0:
                for g in range(4):
                    a = ws.tile([P, HT], F32, tag=f"act{g}",
                                name=f"act{g}_{t}")
                    a3 = a.rearrange("p (m j) -> p m j", m=KH)
                    nc.scalar.activation(
                        out=a3, in_=xg3[:, g * KH:(g + 1) * KH,
                                        t * BLOC:(t + 1) * BLOC],
                        func=(TANH if g == 2 else SIG))
                    acts.append(a)
            else:
                # two psum halves: (i,f) then (g,o)
                pshs = [psmm.tile([P, 2 * HT], F32, tag=f"psL{half}",
                                  name=f"psL{half}_{t}", bufs=2)
                        for half in range(2)]
                for m in range(MC):
                    half, mm = divmod(m, MC // 2)
                    ph = pshs[half]
                    for k in range(KH):
                        nc.tensor.matmul(
                            ph[:, mm * BLOC:(mm + 1) * BLOC],
                            lhsT=whh_sb[:, k * G + m * P:k * G + (m + 1) * P],
                            rhs=h_prev[:, k * BLOC:(k + 1) * BLOC],
                            start=(k == 0), stop=(k == KH - 1))
                for g in range(4):
                    ph3 = pshs[g // 2].rearrange("p (m j) -> p m j",
                                                 m=MC // 2)
                    pre = ws.tile([P, HT], F32, tag=f"pre{g}",
                                  name=f"pre{g}_{t}")
                    pre3 = pre.rearrange("p (m j) -> p m j", m=KH)
                    nc.vector.tensor_add(
                        pre3,
                        ph3[:, (g % 2) * KH:(g % 2 + 1) * KH, :],
                        xg3[:, g * KH:(g + 1) * KH,
                            t * BLOC:(t + 1) * BLOC])
                    a = ws.tile([P, HT], F32, tag=f"act{g}",
                                name=f"act{g}_{t}")
                    nc.scalar.activation(out=a, in_=pre,
                                         func=(TANH if g == 2 else SIG))
                    acts.append(a)
            i_a, f_a, g_a, o_a = acts
            ig = ws.tile([P, HT], F32, tag="ig", name=f"ig_{t}")
            nc.vector.tensor_mul(ig, i_a, g_a)
            if t == 0:
                nc.vector.tensor_copy(out=cT, in_=ig)
            else:
                fc = ws.tile([P, HT], F32, tag="fc", name=f"fc_{t}")
                nc.vector.tensor_mul(fc, f_a, cT)
                nc.vector.tensor_add(cT, ig, fc)
            tc_t = ws.tile([P, HT], F32, tag="tanhc", name=f"tanhc_{t}")
            nc.scalar.activation(out=tc_t, in_=cT, func=TANH)
            h_new = h16pool.tile([P, HT], F16, tag="h16", name=f"h16_{t}")
            nc.vector.tensor_mul(h_new, o_a, tc_t)
            h_prev = h_new

        # ---- head: pred = hT.T @ lin_wT + lin_b ----
        if h_prev is None:
            h_prev = h16pool.tile([P, HT], F16, tag="h16", name="h16_d")
            nc.vector.memset(h_prev, 0.0)
        psP = psmm.tile([BLOC, 1], F32, tag="psL1", name="psP", bufs=2)
        for k in range(KH):
            nc.tensor.matmul(psP, lhsT=h_prev[:, k * BLOC:(k + 1) * BLOC],
                             rhs=linw_sb[:, k:k + 1],
                             start=(k == 0), stop=(k == KH - 1))
        pred_sb = const.tile([BLOC, 1], F32, name="pred_sb", tag="pred_sb")
        nc.scalar.activation(out=pred_sb, in_=psP, func=IDENT,
                             bias=linb_sb, scale=1.0)
        nc.sync.dma_start(out=d_pred[:], in_=pred_sb)


def _prep_in_maps(v, t, W_down, Wt_up_w, Wt_up_b, W_vt, W_ih, W_hh,
                  b_ih, b_hh, lin_w, lin_b):
    """Host-side shard/layout/dtype prep. Layout + cast only, no math."""
    WdT = np.zeros((NPAD, HIN), np.float16)
    WdT[:N] = np.ascontiguousarray(W_down.T).astype(np.float16)
    WvtT = np.ascontiguousarray(W_vt.T).astype(np.float16)
    WihT = np.ascontiguousarray(W_ih.T).astype(np.float16)
    WhhT = np.ascontiguousarray(W_hh.T).astype(np.float16)
    wt_row = np.ascontiguousarray(Wt_up_w.reshape(1, HT)).astype(np.float32)
    wtb_row = np.ascontiguousarray(Wt_up_b.reshape(1, HT)).astype(np.float32)
    bias_g = np.ascontiguousarray(
        (b_ih + b_hh).astype(np.float32).reshape(MC, P).T)
    lin_wT = np.ascontiguousarray(
        lin_w.reshape(KH, P).T).astype(np.float16)
    lin_b_col = np.full((BLOC, 1), np.float32(lin_b[0]), np.float32)

    shared = dict(WdT=WdT, WvtT=WvtT, WihT=WihT, WhhT=WhhT, wt_row=wt_row,
                  wtb_row=wtb_row, bias_g=bias_g, lin_wT=lin_wT,
                  lin_b_col=lin_b_col)
    in_maps = []
    for c in range(NCORES):
        b0 = c * BLOC
        # rows r = l*16 + b  (l-major)
        vr = v[b0:b0 + BLOC].transpose(1, 0, 2).reshape(R, N)
        vT = np.zeros((NPAD, R), np.float16)
        vT[:N] = vr.T.astype(np.float16)
        t_row = np.ascontiguousarray(
            t[b0:b0 + BLOC].T.reshape(1, R)).astype(np.float32)
        in_maps.append(dict(vT=vT, t_row=t_row, **shared))
    return in_maps


def kernel(**inputs):
    nc = _build()
    in_maps = _prep_in_maps(**inputs)
    res = run_bass_kernel_spmd(nc, in_maps, core_ids=list(range(NCORES)))
    return np.concatenate([res.results[c]["pred"] for c in range(NCORES)],
                          axis=0).astype(np.float32)


# revision 8
# speedup vs baseline: 11.1196x; 11.1196x over previous
"""Trainium2 Bass kernel for nn_Discriminator (W_down projection + time-embed
+ W_vt/W_ih projections + 16-step LSTM + linear head).

Strategy: pure data-parallel over batch B=128 across 8 NeuronCores (16
batches/core), all weights replicated. Heavy operands are pre-transposed,
padded, and cast to fp16 on host (layout/precision prep only — all contractions
run on-device with fp32 PSUM accumulation). Everything on-device is laid out
feature-on-partition ("T layout") so the LSTM elementwise ops run on full 128
partitions:

  vT[n, r]      : [20480, 256]  r = l*16 + b (l-major rows per core)
  vdT = WdT.T @ vT   (contraction n, accumulated over 160 K-tiles)
  teT = wt (x) t + wtb (rank-1 matmuls, K=1)
  inputsT = WvtT.T @ [vdT; teT]
  xgT = WihT.T @ inputsT + (b_ih + b_hh)      [4096, 256] fp16, SBUF resident
  LSTM (weight-stationary, gates transposed):
     gatesT_t = xgT[:, t] + sum_k WhhT[k].T @ hT[k]  in two [128,256] PSUM halves
     i,f,g,o on [128,128] tiles; cT/hT as [128, 8*16]
  pred = hT.T @ lin_wT + lin_b  -> [16, 1] per core
"""

import numpy as np

import concourse.bass as bass
import concourse.bacc as bacc
import concourse.tile as tile
from concourse import mybir
from concourse.bass_utils import run_bass_kernel_spmd

F32 = mybir.dt.float32
F16 = mybir.dt.float16

B, L, N = 128, 16, 20000
HIN, H, HT = 512, 1024, 128
G = 4 * H                     # 4096 gate rows
NCORES = 8
BLOC = B // NCORES            # 16 batches per core
R = BLOC * L                  # 256 rows per core (l-major)
P = 128
NCH = 10                      # contraction tiles per DMA chunk
NT = ((N + P - 1) // P + NCH - 1) // NCH * NCH   # 160 contraction tiles
NCHUNKS = NT // NCH           # 20 DMA chunks
NPAD = NT * P                 # 20480
DT = HIN // P                 # 4 vd tiles
KVT = (HIN + HT) // P         # 5 vt contraction tiles
MC = G // P                   # 32 gate row-tiles
KC = HIN // P                 # 4 xg contraction tiles
KH = H // P                   # 8 lstm contraction tiles

_CACHE = {}


def _build(phases=("A", "B", "C", "L"), null=False, reps=1, cache=True):
    """Build + compile the SPMD Bass module once.

    phases/null/reps are ablation & timing knobs for performance experiments;
    the graded path always builds the full kernel with reps=1.
    """
    key = (tuple(phases), null, reps)
    if cache and key in _CACHE:
        return _CACHE[key]

    nc = bacc.Bacc("TRN2", target_bir_lowering=False, debug=False,
                   num_devices=NCORES)

    d_vT = nc.dram_tensor("vT", [NPAD, R], F16, kind="ExternalInput")
    d_WdT = nc.dram_tensor("WdT", [NPAD, HIN], F16, kind="ExternalInput")
    d_WvtT = nc.dram_tensor("WvtT", [HIN + HT, HIN], F16, kind="ExternalInput")
    d_WihT = nc.dram_tensor("WihT", [HIN, G], F16, kind="ExternalInput")
    d_WhhT = nc.dram_tensor("WhhT", [H, G], F16, kind="ExternalInput")
    d_t = nc.dram_tensor("t_row", [1, R], F32, kind="ExternalInput")
    d_wt = nc.dram_tensor("wt_row", [1, HT], F32, kind="ExternalInput")
    d_wtb = nc.dram_tensor("wtb_row", [1, HT], F32, kind="ExternalInput")
    d_bias = nc.dram_tensor("bias_g", [P, MC], F32, kind="ExternalInput")
    d_linw = nc.dram_tensor("lin_wT", [P, KH], F16, kind="ExternalInput")
    d_linb = nc.dram_tensor("lin_b_col", [BLOC, 1], F32, kind="ExternalInput")
    d_pred = nc.dram_tensor("pred", [BLOC, 1], F32, kind="ExternalOutput")

    SIG = mybir.ActivationFunctionType.Sigmoid
    TANH = mybir.ActivationFunctionType.Tanh
    IDENT = mybir.ActivationFunctionType.Identity

    with tile.TileContext(nc) as tc:
        with (
            tc.tile_pool(name="const", bufs=1) as const,
            tc.tile_pool(name="vstream", bufs=3) as vpool,
            tc.tile_pool(name="wdstream", bufs=3) as wdpool,
            tc.tile_pool(name="ws", bufs=2) as ws,
            tc.tile_pool(name="h16", bufs=2) as h16pool,
            tc.tile_pool(name="psmm", bufs=1, space="PSUM") as psmm,
        ):
            if null:
                linb_sb = const.tile([BLOC, 1], F32)
                nc.sync.dma_start(out=linb_sb, in_=d_linb[:])
                pred_sb = const.tile([BLOC, 1], F32)
                nc.scalar.copy(out=pred_sb, in_=linb_sb)
                nc.sync.dma_start(out=d_pred[:], in_=pred_sb)
            else:
                _emit(nc, tc, const, vpool, wdpool, ws, h16pool, psmm,
                      phases, reps,
                      d_vT, d_WdT, d_WvtT, d_WihT, d_WhhT, d_t, d_wt, d_wtb,
                      d_bias, d_linw, d_linb, d_pred, SIG, TANH, IDENT)

    nc.compile()
    if cache:
        _CACHE[key] = nc
    return nc


def _emit(nc, tc, const, vpool, wdpool, ws, h16pool, psmm, phases, reps,
          d_vT, d_WdT, d_WvtT, d_WihT, d_WhhT, d_t, d_wt, d_wtb,
          d_bias, d_linw, d_linb, d_pred, SIG, TANH, IDENT):
    # ---- early constants (small) ----
    wvt_sb = const.tile([P, KVT * HIN], F16)   # 5KB/part
    for k in range(KVT):
        nc.sync.dma_start(out=wvt_sb[:, k * HIN:(k + 1) * HIN],
                          in_=d_WvtT[k * P:(k + 1) * P, :])
    bias_sb = const.tile([P, MC], F32)
    nc.sync.dma_start(out=bias_sb, in_=d_bias[:])
    linw_sb = const.tile([P, KH], F16)
    nc.sync.dma_start(out=linw_sb, in_=d_linw[:])
    linb_sb = const.tile([BLOC, 1], F32)
    nc.sync.dma_start(out=linb_sb, in_=d_linb[:])
    t_sb = const.tile([1, R], F32)
    nc.sync.dma_start(out=t_sb, in_=d_t[:])
    wt_sb = const.tile([1, HT], F32)
    nc.sync.dma_start(out=wt_sb, in_=d_wt[:])
    wtb_sb = const.tile([1, HT], F32)
    nc.sync.dma_start(out=wtb_sb, in_=d_wtb[:])
    ones_sb = const.tile([1, R], F32)
    nc.vector.memset(ones_sb, 1.0)

    vt_sb = const.tile([P, KVT * R], F16)      # vdT + teT
    inpT_sb = const.tile([P, KC * R], F16)     # inputsT
    xgT_sb = const.tile([P, MC * R], F16)      # 16KB/part
    cT = const.tile([P, HT], F32)              # cell state [128, 8*16]

    whh_sb = const.tile([P, KH * G], F16)      # 64KB/part
    wih_sb = const.tile([P, KC * G], F16)      # 32KB/part

    wdT3 = d_WdT.rearrange("(a p) d -> p a d", p=P)   # [128,NT,512]
    vT3 = d_vT.rearrange("(a p) r -> p a r", p=P)     # [128,NT,256]
    xg3 = xgT_sb.rearrange("p (m r) -> p m r", m=MC)  # [128, 32, 256]

    for rep in range(reps):
        # ---- phase A: vdT[d, r] += WdT[n,d].T @ vT[n,r] ----
        # 8 K-tiles per DMA (HWDGE per-dma_start cost amortized)
        psA = [psmm.tile([P, R], F32, tag=f"psA{d}", name=f"psA{d}")
               for d in range(DT)]
        for c in range(NCHUNKS if "A" in phases else 0):
            wd_t = wdpool.tile([P, NCH, HIN], F16, tag="wd",
                               name="wd_t", bufs=3)
            nc.sync.dma_start(out=wd_t,
                              in_=wdT3[:, c * NCH:(c + 1) * NCH, :])
            v_t = vpool.tile([P, NCH, R], F16, tag="v", name="v_t",
                             bufs=3)
            nc.sync.dma_start(out=v_t,
                              in_=vT3[:, c * NCH:(c + 1) * NCH, :])
            for a in range(NCH):
                n = c * NCH + a
                for d in range(DT):
                    nc.tensor.matmul(
                        psA[d], lhsT=wd_t[:, a, d * P:(d + 1) * P],
                        rhs=v_t[:, a, :],
                        start=(n == 0), stop=(n == NT - 1))

        if rep == 0:
            # late preloads: needed from phase C onward; emitted after the
            # phase-A stream so they fill DMA gaps instead of delaying it
            for k in range(KH):
                nc.sync.dma_start(out=whh_sb[:, k * G:(k + 1) * G],
                                  in_=d_WhhT[k * P:(k + 1) * P, :])
            for k in range(KC):
                nc.sync.dma_start(out=wih_sb[:, k * G:(k + 1) * G],
                                  in_=d_WihT[k * P:(k + 1) * P, :])

        # te: wt[j] * t[r] + wtb[j]  (rank-1 updates, K=1)
        psTE = psmm.tile([P, R], F32, tag="psL0", name="psTE", bufs=2)
        nc.tensor.matmul(psTE, lhsT=wt_sb, rhs=t_sb, start=True, stop=False)
        nc.tensor.matmul(psTE, lhsT=wtb_sb, rhs=ones_sb, start=False,
                         stop=True)
        if "A" in phases:
            for d in range(DT):
                if d % 2 == 0:
                    nc.vector.tensor_copy(out=vt_sb[:, d * R:(d + 1) * R],
                                          in_=psA[d])
                else:
                    nc.scalar.copy(out=vt_sb[:, d * R:(d + 1) * R],
                                   in_=psA[d])
        nc.vector.tensor_copy(out=vt_sb[:, DT * R:(DT + 1) * R], in_=psTE)

        # ---- phase B: inputsT[i, r] ----
        for m in range(KC if "B" in phases else 0):
            psB = psmm.tile([P, R], F32, tag=f"psA{m}", name=f"psB{m}")
            for k in range(KVT):
                nc.tensor.matmul(
                    psB,
                    lhsT=wvt_sb[:, k * HIN + m * P:k * HIN + (m + 1) * P],
                    rhs=vt_sb[:, k * R:(k + 1) * R],
                    start=(k == 0), stop=(k == KVT - 1))
            nc.vector.tensor_copy(out=inpT_sb[:, m * R:(m + 1) * R],
                                  in_=psB)

        # ---- phase C: xgT[g, r] = WihT.T @ inputsT + bias ----
        for m in range(MC if "C" in phases else 0):
            psC = psmm.tile([P, R], F32, tag=f"psA{m % DT}", name=f"psC{m}")
            for k in range(KC):
                nc.tensor.matmul(
                    psC,
                    lhsT=wih_sb[:, k * G + m * P:k * G + (m + 1) * P],
                    rhs=inpT_sb[:, k * R:(k + 1) * R],
                    start=(k == 0), stop=(k == KC - 1))
            if m % 2 == 0:
                nc.scalar.activation(out=xgT_sb[:, m * R:(m + 1) * R],
                                     in_=psC, func=IDENT,
                                     bias=bias_sb[:, m:m + 1], scale=1.0)
            else:
                nc.vector.tensor_scalar_add(xgT_sb[:, m * R:(m + 1) * R],
                                            psC, bias_sb[:, m:m + 1])

        # ---- LSTM over L steps ----
        h_prev = None
        for t in range(L if "L" in phases else 0):
            acts = []
            if t == 0:
                for g in range(4):
                    a = ws.tile([P, HT], F32, tag=f"act{g}",
                                name=f"act{g}_{t}")
                    a3 = a.rearrange("p (m j) -> p m j", m=KH)
                    nc.scalar.activation(
                        out=a3, in_=xg3[:, g * KH:(g + 1) * KH,
                                        t * BLOC:(t + 1) * BLOC],
                        func=(TANH if g == 2 else SIG))
                    acts.append(a)
            else:
                # two psum halves: (i,f) then (g,o)
                pshs = [psmm.tile([P, 2 * HT], F32, tag=f"psL{half}",
                                  name=f"psL{half}_{t}", bufs=2)
                        for half in range(2)]
                # emit (i,g) into psL0 first, then (f,o) into psL1 — c's
                # dependencies retire before o's matmuls finish
                order = [0, 2, 1, 3]           # i, g, f, o
                for gi, g in enumerate(order):
                    half, pos = divmod(gi, 2)
                    ph = pshs[half]
                    for mt in range(KH):
                        m = g * KH + mt
                        mm = pos * KH + mt
                        for k in range(KH):
                            nc.tensor.matmul(
                                ph[:, mm * BLOC:(mm + 1) * BLOC],
                                lhsT=whh_sb[:, k * G + m * P:
                                            k * G + (m + 1) * P],
                                rhs=h_prev[:, k * BLOC:(k + 1) * BLOC],
                                start=(k == 0), stop=(k == KH - 1))
                acts = [None] * 4
                for gi, g in enumerate(order):
                    half, pos = divmod(gi, 2)
                    ph3 = pshs[half].rearrange("p (m j) -> p m j", m=MC // 2)
                    pre = ws.tile([P, HT], F32, tag=f"pre{g}",
                                  name=f"pre{g}_{t}")
                    pre3 = pre.rearrange("p (m j) -> p m j", m=KH)
                    nc.vector.tensor_add(
                        pre3,
                        ph3[:, pos * KH:(pos + 1) * KH, :],
                        xg3[:, g * KH:(g + 1) * KH,
                            t * BLOC:(t + 1) * BLOC])
                    a = ws.tile([P, HT], F32, tag=f"act{g}",
                                name=f"act{g}_{t}")
                    nc.scalar.activation(out=a, in_=pre,
                                         func=(TANH if g == 2 else SIG))
                    acts[g] = a
            i_a, f_a, g_a, o_a = acts
            ig = ws.tile([P, HT], F32, tag="ig", name=f"ig_{t}")
            nc.vector.tensor_mul(ig, i_a, g_a)
            if t == 0:
                nc.vector.tensor_copy(out=cT, in_=ig)
            else:
                fc = ws.tile([P, HT], F32, tag="fc", name=f"fc_{t}")
                nc.vector.tensor_mul(fc, f_a, cT)
                nc.vector.tensor_add(cT, ig, fc)
            tc_t = ws.tile([P, HT], F32, tag="tanhc", name=f"tanhc_{t}")
            nc.scalar.activation(out=tc_t, in_=cT, func=TANH)
            h_new = h16pool.tile([P, HT], F16, tag="h16", name=f"h16_{t}")
            nc.vector.tensor_mul(h_new, o_a, tc_t)
            h_prev = h_new

        # ---- head: pred = hT.T @ lin_wT + lin_b ----
        if h_prev is None:
            h_prev = h16pool.tile([P, HT], F16, tag="h16", name="h16_d")
            nc.vector.memset(h_prev, 0.0)
        psP = psmm.tile([BLOC, 1], F32, tag="psL1", name="psP", bufs=2)
        for k in range(KH):
            nc.tensor.matmul(psP, lhsT=h_prev[:, k * BLOC:(k + 1) * BLOC],
                             rhs=linw_sb[:, k:k + 1],
                             start=(k == 0), stop=(k == KH - 1))
        pred_sb = const.tile([BLOC, 1], F32, name="pred_sb", tag="pred_sb")
        nc.scalar.activation(out=pred_sb, in_=psP, func=IDENT,
                             bias=linb_sb, scale=1.0)
        nc.sync.dma_start(out=d_pred[:], in_=pred_sb)


def _prep_in_maps(v, t, W_down, Wt_up_w, Wt_up_b, W_vt, W_ih, W_hh,
                  b_ih, b_hh, lin_w, lin_b):
    """Host-side shard/layout/dtype prep. Layout + cast only, no math."""
    WdT = np.zeros((NPAD, HIN), np.float16)
    WdT[:N] = np.ascontiguousarray(W_down.T).astype(np.float16)
    WvtT = np.ascontiguousarray(W_vt.T).astype(np.float16)
    WihT = np.ascontiguousarray(W_ih.T).astype(np.float16)
    WhhT = np.ascontiguousarray(W_hh.T).astype(np.float16)
    wt_row = np.ascontiguousarray(Wt_up_w.reshape(1, HT)).astype(np.float32)
    wtb_row = np.ascontiguousarray(Wt_up_b.reshape(1, HT)).astype(np.float32)
    bias_g = np.ascontiguousarray(
        (b_ih + b_hh).astype(np.float32).reshape(MC, P).T)
    lin_wT = np.ascontiguousarray(
        lin_w.reshape(KH, P).T).astype(np.float16)
    lin_b_col = np.full((BLOC, 1), np.float32(lin_b[0]), np.float32)

    shared = dict(WdT=WdT, WvtT=WvtT, WihT=WihT, WhhT=WhhT, wt_row=wt_row,
                  wtb_row=wtb_row, bias_g=bias_g, lin_wT=lin_wT,
                  lin_b_col=lin_b_col)
    in_maps = []
    for c in range(NCORES):
        b0 = c * BLOC
        # rows r = l*16 + b  (l-major)
        vr = v[b0:b0 + BLOC].transpose(1, 0, 2).reshape(R, N)
        vT = np.zeros((NPAD, R), np.float16)
        vT[:N] = vr.T.astype(np.float16)
        t_row = np.ascontiguousarray(
            t[b0:b0 + BLOC].T.reshape(1, R)).astype(np.float32)
        in_maps.append(dict(vT=vT, t_row=t_row, **shared))
    return in_maps


def kernel(**inputs):
    nc = _build()
    in_maps = _prep_in_maps(**inputs)
    res = run_bass_kernel_spmd(nc, in_maps, core_ids=list(range(NCORES)))
    return np.concatenate([res.results[c]["pred"] for c in range(NCORES)],
                          axis=0).astype(np.float32)
